# revision 67
# baseline (speedup 1.0000x reference)
"""Trainium2 Bass kernel for nn_MultiHeadAttention_30846455119878.

8-core strategy (unchanged from v1):
  - Attention head-sharded: core m owns heads {2m, 2m+1}; q/k/v projections for
    its 2 heads over all B*T tokens, causal softmax attention per (batch, head).
  - Per-batch AllToAll re-shards heads -> tokens; each core then runs the full
    output projection for its 1/8 token slice of every batch.
  - Host: x passed pre-transposed as x^T [C, B*T] bf16; matmuls bf16 -> fp32.

v2 rewrite: software-pipelined schedule. v1 ran phases serially per batch
(A=proj, B=attention, C=outproj at end); B is ScalarE(exp)-paced so PE idled
~30% there, plus 4x ~7.4us PE gaps at batch bounds (xt DMA for b+1 queued
behind the den/recip DMA chain on the sync queue) and a ~26us exposed tail.
v2 interleaves A(b+1) and C(b-1) PE work into B(b)'s sb-loop as "filler"
groups so PE never waits on ACT, and:
  - paired-head score PSUM tile [128, 2, 512] -> ONE exp per key-block
    (halves ACT instruction overhead; ~28us)
  - inner loop emits scores(sb+1) before attV(sb) so attV's et wait is hidden
  - staging DMAs moved ACT -> GPSIMD queue (v1: 52us of DMA_DIRECT2D on ACT)
  - den row copied out of PSUM by the idle GPSIMD/Pool engine, not DVE
  - rb (1/den broadcast) PSUM->SBUF copies on DVE, not ACT
  - per-batch dedicated rcv buffers (no cross-engine WAR on the collective
    read-back; v1's inlined-outproj attempt failed on HW, suspected multi-sem
    wait hazard -- deps kept to cc only)
  - out writes on the DVE queue, xt/rcv/den-reshape on sync, weights split
    sync/vector/gpsimd so the t=0 load chain is parallel.

Perf log (HW, 8 axon trn2 cores):
  - v1 baseline: 470728 / 434143 ns (run variance), rel 3.8e-3.
    Trace: PE busy 355us (MATMUL union; LDWEIGHTS fully hidden), ACT 280us
    (227 ACTIVATE + 52 DMA), DVE 153us. throttle_active 235us of 440us!
    PE gaps only 63us (xt-at-batch-boundary + tail).
  - v2 (this file): see test runs.
Known-broken paths (do not retry): reciprocal_approx_fast / gpsimd
partition_broadcast (garbage via bass2jax), XBAR dma_start_transpose with
strided src, scalar-engine Reciprocal/Rsqrt (banned in bass).
"""

import sys

if "/opt/trn_rl_repo" not in sys.path:
    sys.path.insert(0, "/opt/trn_rl_repo")

import numpy as np
import ml_dtypes

import concourse.bass as bass
import concourse.tile as tile
from concourse import bacc, mybir
from concourse.bass_utils import run_bass_kernel_spmd
from concourse.tile_rust import add_dep_helper

BF16 = ml_dtypes.bfloat16
E4M3 = ml_dtypes.float8_e4m3

# fp8e4 DoubleRow score matmuls: HW-measured SLOWER (DR streams 2N moving rows
# at 1 row/cyc -- the cost model's 0.5 cyc/row is wrong for K<=64). Keep off.
FP8_SCORES = False
# plain fp8 q/k scores: same row count as bf16, but narrower multipliers ease
# the power throttle (HW: 385 -> 368us). FP8_PROJ additionally runs the q/k
# PROJECTIONS in fp8 (x8 and 32*Wq/32*Wk as fp8; the 32x scaling keeps the
# tiny weights out of e4m3's subnormal range, folded back via the exp scale).
FP8_PLAIN = False
FP8_PROJ = False
WSCALE = 32.0

# Full problem dims
B_FULL, T_FULL, C_FULL, H_FULL, D_HEAD = 4, 2048, 1024, 16, 64
N_CORES = 8
HPC = H_FULL // N_CORES  # heads per core = 2
F = HPC * D_HEAD         # per-core attention feature rows = 128
TCH = 512                # query-chunk (free dim of score matmuls)
D = D_HEAD


def build_nc(B=B_FULL, T=T_FULL, C=C_FULL):
    """Build the SPMD Bass graph (same graph on all 8 cores)."""
    dt = mybir.dt
    CK = C // 128        # contraction chunks for projections = 8
    NTC = T // TCH       # query chunks per sequence = 4
    NSB = T // 128       # key blocks per sequence = 16
    SBB = TCH // 128     # key blocks per query chunk diagonal = 4
    TS = T // N_CORES    # token shard per (batch, core) = 256
    CO = H_FULL * D_HEAD  # output feature dim = 1024
    TT = 128             # token tile for output projection
    scale = float(1.0 / np.sqrt(C))
    if FP8_PROJ:
        scale /= float(WSCALE * WSCALE)  # q,k carry a WSCALE factor each

    qk_wdt = dt.float8e4 if FP8_PROJ else dt.bfloat16
    nc = bacc.Bacc()
    xt_d = nc.declare_dram_parameter("xt", [128, CK, B * T], dt.bfloat16, isOutput=False)
    if FP8_PROJ:
        # fp8 copy of x^T for the q/k projections (v stays bf16)
        x8_d = nc.declare_dram_parameter("x8", [128, CK, B * T], dt.float8e4, isOutput=False)
    wq_d = nc.declare_dram_parameter("wq", [128, CK, F], qk_wdt, isOutput=False)
    wk_d = nc.declare_dram_parameter("wk", [128, CK, F], qk_wdt, isOutput=False)
    wv_d = nc.declare_dram_parameter("wv", [128, CK, F], dt.bfloat16, isOutput=False)
    wo_d = nc.declare_dram_parameter("wo", [128, N_CORES, CO], dt.bfloat16, isOutput=False)
    bo_d = nc.declare_dram_parameter("bo", [1, CO], dt.bfloat16, isOutput=False)
    mask_d = nc.declare_dram_parameter("mask", [128, SBB, TCH], dt.bfloat16, isOutput=False)
    out_d = nc.declare_dram_parameter("out", [B, TS, CO], dt.float32, isOutput=True)

    # one collective per (batch, half): core m owns tokens
    # {e*1024 + m*128 + t} so half e's payload is complete after query
    # chunks 2e..2e+1 -- cc_a fires MID-batch, shrinking the exposed tail
    TH = T // (2 * N_CORES)  # tokens per (core, half) = 128
    cc_in = [[nc.dram_tensor(f"cc_in{b}_{e}", [N_CORES, F, TH], dt.bfloat16)
              for e in range(2)] for b in range(B)]
    cc_out = [[nc.dram_tensor(f"cc_out{b}_{e}", [N_CORES, F, TH], dt.bfloat16)
               for e in range(2)] for b in range(B)]
    rg = [list(range(N_CORES))]

    with tile.TileContext(nc) as tc:
        from contextlib import ExitStack

        with ExitStack() as ctx:
            wpool = ctx.enter_context(tc.tile_pool(name="w", bufs=1))
            xpool = ctx.enter_context(tc.tile_pool(name="xt", bufs=3))
            qkpool = ctx.enter_context(tc.tile_pool(name="qk", bufs=2))
            v1pool = ctx.enter_context(tc.tile_pool(name="v1", bufs=2))
            epool = ctx.enter_context(tc.tile_pool(name="exp", bufs=6))
            aupool = ctx.enter_context(tc.tile_pool(name="attu", bufs=2))
            recpool = ctx.enter_context(tc.tile_pool(name="rec", bufs=1))
            rbpool = ctx.enter_context(tc.tile_pool(name="rb", bufs=3))
            atpool = ctx.enter_context(tc.tile_pool(name="attn", bufs=2))
            rcvpool = ctx.enter_context(tc.tile_pool(name="rcv", bufs=1))
            outpool = ctx.enter_context(tc.tile_pool(name="osb", bufs=2))
            psS = ctx.enter_context(tc.tile_pool(name="psS", bufs=2, space="PSUM"))
            psB = ctx.enter_context(tc.tile_pool(name="psB", bufs=1, space="PSUM"))
            psP = ctx.enter_context(tc.tile_pool(name="psP", bufs=2, space="PSUM"))

            # ---- resident constants; split the preload across queues so the
            # first A(0) matmuls start ~4us in (v1: 15.7us serial-load gap).
            wq_sb = wpool.tile([128, CK, F], qk_wdt, tag="wq")
            wk_sb = wpool.tile([128, CK, F], qk_wdt, tag="wk")
            wv_sb = wpool.tile([128, CK, F], dt.bfloat16, tag="wv")
            wo_sb = wpool.tile([128, N_CORES, CO], dt.bfloat16, tag="wo")
            bo_sb = wpool.tile([1, CO], dt.bfloat16, tag="bo")
            mask_sb = wpool.tile([128, SBB, TCH], dt.bfloat16, tag="mask")
            ones_sb = wpool.tile([1, 128], dt.bfloat16, tag="ones")
            # first A(0,0) matmul needs x chunk 0 AND wq: load those first
            xt00 = xpool.tile([128, CK, TCH], dt.bfloat16, tag="xt", name="xt00")
            x800 = None
            if FP8_PROJ:
                x800 = xpool.tile([128, CK, TCH], dt.float8e4, tag="x8", name="x800")
                nc.sync.dma_start(out=x800, in_=x8_d[:, :, 0:TCH])
            nc.sync.dma_start(out=wq_sb, in_=wq_d[:, :, :])
            nc.sync.dma_start(out=xt00, in_=xt_d[:, :, 0:TCH])
            nc.sync.dma_start(out=wk_sb, in_=wk_d[:, :, :])
            nc.gpsimd.dma_start(out=wv_sb, in_=wv_d[:, :, :])
            # defer the bulky mask/wo/bo loads (2.5MB) until the critical
            # xt00+wq transfers are done -- concurrent preloads saturate HBM
            # and pushed the first matmul from ~15us to ~30us
            deferred_loads = [
                nc.gpsimd.dma_start(out=mask_sb, in_=mask_d[:, :, :]).ins,
                nc.gpsimd.dma_start(out=wo_sb, in_=wo_d[:, :, :]).ins,
                nc.gpsimd.dma_start(out=bo_sb, in_=bo_d[:, :]).ins,
            ]
            nc.vector.memset(ones_sb, 1.0)

            # per-batch persistent tiles, filled lazily
            qT = {}
            kT = {}
            qp = {}
            kp = {}
            v1 = {}
            att_un = {}
            attn = {}
            den_t = {}
            rec_t = {}
            rec_all = {}
            rcv = {}
            cc_insts = {}
            stg = {}

            filler = []   # queued (callable) PE work groups from A/C phases
            normq = []    # norm groups: drained only when filler is empty
                          # (their rb matmuls trail the den->recip DMA chain)

            def drain(n):
                for _ in range(n):
                    if filler:
                        filler.pop(0)()
                    elif normq:
                        normq.pop(0)()
                    else:
                        return

            # ---------- phase A: q/k/v projections for batch b, chunk tcb ----
            def prep_A_chunk(b, tcb):
                qk_dt = dt.float8e4 if (FP8_SCORES or FP8_PLAIN) else dt.bfloat16
                if tcb == 0:
                    qT[b] = qkpool.tile([F, T], qk_dt, tag="qT", name=f"qT{b}")
                    kT[b] = qkpool.tile([F, T], qk_dt, tag="kT", name=f"kT{b}")
                    if FP8_SCORES:
                        qp[b] = qkpool.tile([64, 2, T], dt.float8e4, tag="qp", name=f"qp{b}")
                        kp[b] = qkpool.tile([64, 2, T], dt.float8e4, tag="kp", name=f"kp{b}")
                    v1[b] = v1pool.tile([128, NSB, HPC, 80], dt.bfloat16, tag="v1", name=f"v1_{b}")
                    nc.vector.memset(v1[b][:, :, :, D:D + 1], 1.0)
                g0 = b * T + tcb * TCH
                if b == 0 and tcb == 0:
                    xt_sb = xt00  # preloaded ahead of the weights
                    x8_sb = x800
                else:
                    xt_sb = xpool.tile([128, CK, TCH], dt.bfloat16, tag="xt")
                    # batch 0 rides the idle scalar queue, overlapping the
                    # sync queue's weight preload at t=0
                    xq = nc.scalar if b == 0 else nc.sync
                    if FP8_PROJ:
                        x8_sb = xpool.tile([128, CK, TCH], dt.float8e4, tag="x8")
                        xq.dma_start(out=x8_sb, in_=x8_d[:, :, g0:g0 + TCH])
                    xq.dma_start(out=xt_sb, in_=xt_d[:, :, g0:g0 + TCH])

                def qk_group(w_sb, dstT, dstP):
                    def go():
                        qk_rhs = x8_sb if FP8_PROJ else xt_sb
                        pp = psP.tile([128, TCH], dt.float32, tag="pp")
                        for o in range(CK):
                            nc.tensor.matmul(
                                pp, lhsT=w_sb[:, o, :], rhs=qk_rhs[:, o, :],
                                start=(o == 0), stop=(o == CK - 1),
                            )
                        cp = nc.vector.tensor_copy(
                            out=dstT[:, tcb * TCH:(tcb + 1) * TCH], in_=pp
                        )
                        if deferred_loads:
                            # release the bulky preloads once startup-critical
                            # transfers are done
                            while deferred_loads:
                                add_dep_helper(deferred_loads.pop(), cp.ins,
                                               sync=True, reason="defer preload")
                        if FP8_SCORES:
                            # repack [128(feat), t] -> [64(h*32+p), 2(r), t] for
                            # DoubleRow fp8 scores: feat = h*64 + r*32 + p.
                            # NOTE: must be plain partition-range DMAs; a
                            # rearranged-AP DMA mismaps partition strides (HW
                            # verified: garbage except partitions 0/32).
                            t0 = tcb * TCH
                            for h in range(2):
                                for r in range(2):
                                    s = h * 64 + r * 32
                                    nc.gpsimd.dma_start(
                                        out=dstP[32 * h:32 * h + 32, r, t0:t0 + TCH],
                                        in_=dstT[s:s + 32, t0:t0 + TCH],
                                    )
                    return go

                def v_group(ssub):
                    def go():
                        # 2 ssubs packed per psP tile (2 indep accum regions)
                        if ssub % 2 == 0:
                            v_group.pp = psP.tile([128, TCH], dt.float32, tag="pp",
                                                  name=f"vpp_{b}_{tcb}_{ssub}")
                        pp = v_group.pp
                        r0 = (ssub % 2) * 128
                        for o in range(CK):
                            nc.tensor.matmul(
                                pp[:, r0:r0 + F],
                                lhsT=xt_sb[:, o, ssub * 128:(ssub + 1) * 128],
                                rhs=wv_sb[:, o, :],
                                start=(o == 0), stop=(o == CK - 1),
                            )
                        st = tcb * SBB + ssub
                        nc.vector.tensor_copy(
                            out=v1[b][:, st, :, 0:D],
                            in_=pp[:, r0:r0 + F].rearrange("p (h d) -> p h d", h=HPC),
                        )
                    return go

                filler.append(qk_group(wq_sb, qT[b], qp.get(b)))
                filler.append(qk_group(wk_sb, kT[b], kp.get(b)))
                for ssub in range(SBB):
                    filler.append(v_group(ssub))

            # ---------- phase C: outproj piece (tt, c2) of batch b -----------
            def prep_C_piece(b, piece):
                def go():
                    tt, c2 = divmod(piece, CO // 512)  # tt == half index e
                    if c2 == 0:
                        rcv[(b, tt)] = rcvpool.tile(
                            [128, N_CORES, TH], dt.bfloat16,
                            tag=f"rcv{b}_{tt}", name=f"rcv{b}_{tt}")
                        r = nc.gpsimd.dma_start(
                            out=rcv[(b, tt)],
                            in_=cc_out[b][tt][:, :, :].rearrange("j p t -> p j t"),
                        )
                        add_dep_helper(r.ins, cc_insts[(b, tt)], sync=True,
                                       reason="cc_out RAW")
                    pp = psP.tile([128, TCH], dt.float32, tag="pp")
                    for j in range(N_CORES):
                        nc.tensor.matmul(
                            pp[0:TT, 0:512],
                            lhsT=rcv[(b, tt)][:, j, :],
                            rhs=wo_sb[:, j, c2 * 512:(c2 + 1) * 512],
                            start=(j == 0), stop=(j == N_CORES - 1),
                        )
                    osb = outpool.tile([TT, 512], dt.float32, tag="osb")
                    # bias folded into the PSUM->SBUF copy (same DVE cost as a
                    # plain copy; replaces a 512-row bias matmul per piece)
                    nc.vector.tensor_tensor(
                        out=osb, in0=pp[0:TT, 0:512], in1=bias_full[:, c2, :],
                        op=mybir.AluOpType.add,
                    )
                    nc.gpsimd.dma_start(
                        out=out_d[b, tt * TT:(tt + 1) * TT, c2 * 512:(c2 + 1) * 512],
                        in_=osb,
                    )
                filler.append(go)

            # ---------- phase B: attention for (b, tcb), fillers interleaved -
            def emit_B(b, tcb):
                nsb = SBB * (tcb + 1)
                att_ps = psB.tile([D + 1, HPC, TCH], dt.float32, tag="att",
                                  name=f"attps_{b}_{tcb}")

                pend = None  # (sb, c0, et) awaiting attV

                def attv(sb, c0, et):
                    for h in range(HPC):
                        nc.tensor.matmul(
                            att_ps[:, h, c0:TCH],
                            lhsT=v1[b][:, sb, h, 0:D + 1],
                            rhs=et[:, h, c0:TCH],
                            start=(sb == 0), stop=(sb == nsb - 1),
                        )

                for sb in range(nsb):
                    j0 = sb - SBB * tcb
                    c0 = j0 * 128 if j0 > 0 else 0
                    s_ps = psS.tile([128, HPC, TCH], dt.float32, tag="sc")
                    for h in range(HPC):
                        if FP8_SCORES:
                            # fp8e4 DoubleRow: K=64 packed [32, 2]; 0.5 cyc/row
                            nc.tensor.matmul(
                                s_ps[:, h, c0:TCH],
                                lhsT=kp[b][32 * h:32 * (h + 1), :, sb * 128:(sb + 1) * 128],
                                rhs=qp[b][32 * h:32 * (h + 1), :, tcb * TCH + c0:(tcb + 1) * TCH],
                                start=True, stop=True,
                                perf_mode=mybir.MatmulPerfMode.DoubleRow,
                                tile_position=(32 * h, 0),
                            )
                        else:
                            nc.tensor.matmul(
                                s_ps[:, h, c0:TCH],
                                lhsT=kT[b][h * D:(h + 1) * D, sb * 128:(sb + 1) * 128],
                                rhs=qT[b][h * D:(h + 1) * D, tcb * TCH + c0:(tcb + 1) * TCH],
                                start=True, stop=True,
                                tile_position=(h * D, 0),
                            )
                    et = epool.tile([128, HPC, TCH], dt.bfloat16, tag="exp")
                    nc.scalar.activation(
                        out=et[:, :, c0:TCH], in_=s_ps[:, :, c0:TCH],
                        func=mybir.ActivationFunctionType.Exp, scale=scale,
                    )
                    if j0 >= 0:
                        for h in range(HPC):
                            nc.vector.tensor_mul(
                                et[:, h, c0:TCH], et[:, h, c0:TCH],
                                mask_sb[:, j0, c0:TCH],
                            )
                    # keep PE ahead of ACT: a filler group between this block's
                    # scores and the previous block's attV hides the et wait
                    drain(1)
                    if pend is not None:
                        attv(*pend)
                    pend = (sb, c0, et)
                attv(*pend)

                # unnormalized attention + denominator out of PSUM in ONE copy
                # (DVE cost is free-size per partition; the extra den row is free)
                nc.vector.tensor_copy(
                    out=att_un[b][:, tcb, :, :], in_=att_ps[:, :, :]
                )
                # per-tcb denominator -> 128-lane reciprocal -> rec_all slice;
                # on sync (not scalar: queued exps would add ~us to the chain,
                # stalling the rb matmuls; xt loads are prefetched a tcb ahead
                # so a <1us den wait ahead of them is harmless)
                nc.sync.dma_start(
                    out=den_t[b][:, tcb * 8:(tcb + 1) * 8],
                    in_=att_un[b][D:D + 1, tcb, :, :],
                )
                with nc.allow_low_precision(reason="bf16 softmax denom recip ok at rel 2e-2"):
                    nc.vector.reciprocal(
                        out=rec_t[b][:, tcb * 8:(tcb + 1) * 8],
                        in_=den_t[b][:, tcb * 8:(tcb + 1) * 8],
                    )
                nc.sync.dma_start(
                    out=rec_all[b][0:1, tcb * HPC * TCH:(tcb + 1) * HPC * TCH],
                    in_=rec_t[b][:, tcb * 8:(tcb + 1) * 8],
                )

                # normalization + staging for this tcb, deferred as filler so
                # the rb matmuls never head-block the PE stream; when a half's
                # last chunk is staged, trigger that half's AllToAll
                def norm_group(tcb=tcb):
                    e = tcb // 2
                    for h in range(HPC):
                        slot = tcb * HPC + h
                        rb_ps = psP.tile([128, TCH], dt.float32, tag="pp",
                                         name=f"rbps_{b}_{slot}")
                        nc.tensor.matmul(
                            rb_ps[0:D, :], lhsT=ones_sb[0:1, 0:D],
                            rhs=rec_all[b][0:1, slot * TCH:(slot + 1) * TCH],
                            start=True, stop=True,
                        )
                        rb_sb = rbpool.tile([D, TCH], dt.bfloat16, tag="recbc")
                        nc.vector.tensor_copy(out=rb_sb, in_=rb_ps[0:D, :])
                        nc.vector.tensor_mul(
                            attn[b][:, h, tcb * TCH:(tcb + 1) * TCH],
                            att_un[b][0:D, tcb, h, :], rb_sb,
                        )
                        for jj in range(SBB):
                            j = (tcb % 2) * SBB + jj  # core index for this slice
                            t0 = tcb * TCH + jj * TH
                            stg[b][e].append(nc.gpsimd.dma_start(
                                out=cc_in[b][e][j, h * D:(h + 1) * D, :],
                                in_=attn[b][:, h, t0:t0 + TH],
                            ).ins)
                    if tcb % 2 == 1:
                        cc = nc.gpsimd.collective_compute(
                            "AllToAll", mybir.AluOpType.bypass, replica_groups=rg,
                            ins=[cc_in[b][e].ap().opt()],
                            outs=[cc_out[b][e].ap().opt()],
                        )
                        for s in stg[b][e]:
                            add_dep_helper(cc.ins, s, sync=True, reason="cc_in RAW")
                        cc_insts[(b, e)] = cc.ins
                normq.append(norm_group)

            # ---------- batch end: drain leftovers (cc fires inside norm) ----
            def emit_norm_cc(b):
                drain(len(filler) + len(normq))

            # ================= main software-pipelined schedule ==============
            for tcb in range(NTC):
                prep_A_chunk(0, tcb)
                drain(len(filler))  # A(0) runs solo up front; drain per chunk
                # (xpool bufs=3: chunk tcb's readers must be emitted before
                # chunk tcb+3's DMA so the pool WAR dep is seen)

            # bias broadcast to all 128 token rows, once (consumed by every
            # outproj piece's fused copy+add)
            bias_full = wpool.tile([128, CO // 512, 512], dt.float32, tag="biasf")
            for c2 in range(CO // 512):
                bp_ps = psP.tile([128, TCH], dt.float32, tag="pp",
                                 name=f"biasps{c2}")
                nc.tensor.matmul(
                    bp_ps, lhsT=ones_sb[0:1, :],
                    rhs=bo_sb[0:1, c2 * 512:(c2 + 1) * 512],
                    start=True, stop=True,
                )
                nc.vector.tensor_copy(out=bias_full[:, c2, :], in_=bp_ps)

            for b in range(B):
                att_un[b] = aupool.tile([D + 1, NTC, HPC, TCH], dt.bfloat16,
                                        tag="attu", name=f"attu{b}")
                attn[b] = atpool.tile([D, HPC, T], dt.bfloat16,
                                      tag="at", name=f"attn{b}")
                den_t[b] = recpool.tile([128, NTC * HPC * TCH // 128], dt.bfloat16,
                                        tag="dent", name=f"dent{b}")
                rec_t[b] = recpool.tile([128, NTC * HPC * TCH // 128], dt.bfloat16,
                                        tag="rect", name=f"rect{b}")
                rec_all[b] = recpool.tile([1, NTC * HPC * TCH], dt.bfloat16,
                                          tag="recall", name=f"recall{b}")
                stg[b] = [[], []]
                last = b == B - 1
                for tcb in range(NTC):
                    if b + 1 < B:
                        # hold back the last batch's final chunk: it isn't
                        # needed until B(last,3), and draining it inside the
                        # otherwise filler-less B(last) keeps PE ahead of ACT
                        if not (b + 1 == B - 1 and tcb == 3):
                            prep_A_chunk(b + 1, tcb)
                    elif tcb == 0:
                        prep_A_chunk(b, 3)
                    if b >= 1 and tcb >= 1 and not (last and tcb == 3):
                        prep_C_piece(b - 1, tcb - 1)
                    emit_B(b, tcb)
                if b >= 1 and not last:
                    prep_C_piece(b - 1, 3)
                if last:
                    # C(B-2) pieces 2-3 are runnable now (their collective is
                    # long done); filler-priority puts them ahead of the final
                    # norm groups' rec-chain-waiting rb matmuls
                    prep_C_piece(b - 1, 2)
                    prep_C_piece(b - 1, 3)
                emit_norm_cc(b)  # drains remaining fillers

            # tail: last batch's outproj
            for piece in range(4):
                prep_C_piece(B - 1, piece)
            drain(len(filler) + len(normq))

    nc.finalize()
    return nc


def prep_inputs(x, Wq, Wk, Wv, Wo, bo):
    """Host-side shard/layout prep. Returns in_maps for the 8 cores."""
    B, T, C = x.shape
    CK = C // 128
    SBB = TCH // 128

    x = np.asarray(x, dtype=np.float32)
    xtf = x.reshape(B * T, C).T  # [C, B*T] fp32
    xt = np.ascontiguousarray(
        xtf.astype(BF16).reshape(CK, 128, B * T).transpose(1, 0, 2))
    if FP8_PROJ:
        x8 = np.ascontiguousarray(
            xtf.astype(E4M3).reshape(CK, 128, B * T).transpose(1, 0, 2))

    CO = Wo.shape[1]
    wo_h = np.ascontiguousarray(
        np.asarray(Wo, np.float32).astype(BF16).reshape(N_CORES, 128, CO).transpose(1, 0, 2)
    )
    bo_h = np.asarray(bo, np.float32).astype(BF16).reshape(1, CO)

    p = np.arange(128)[:, None, None]
    j = np.arange(SBB)[None, :, None]
    t = np.arange(TCH)[None, None, :]
    mask_h = (t >= p + j * 128).astype(BF16)

    in_maps = []
    for m in range(N_CORES):
        maps = {"xt": xt, "wo": wo_h, "bo": bo_h, "mask": mask_h}
        if FP8_PROJ:
            maps["x8"] = x8
        for name, W in (("wq", Wq), ("wk", Wk), ("wv", Wv)):
            Ws = np.concatenate(
                [np.asarray(W[HPC * m + i], np.float32) for i in range(HPC)], axis=1
            )  # [C, F]
            if FP8_PROJ and name in ("wq", "wk"):
                # x32 lifts the tiny (~0.02 std) weights out of e4m3's
                # subnormal range; folded back via the exp scale
                maps[name] = np.ascontiguousarray(
                    (Ws * WSCALE).astype(E4M3).reshape(CK, 128, F).transpose(1, 0, 2)
                )
            else:
                maps[name] = np.ascontiguousarray(
                    Ws.astype(BF16).reshape(CK, 128, F).transpose(1, 0, 2)
                )
        in_maps.append(maps)
    return in_maps


_NC_CACHE = {}


def _get_nc(B, T, C):
    key = (B, T, C)
    if key not in _NC_CACHE:
        _NC_CACHE[key] = build_nc(B, T, C)
    return _NC_CACHE[key]


def kernel(x, Wq, Wk, Wv, Wo, bo, _trace=False):
    x = np.asarray(x)
    B, T, C = x.shape
    nc = _get_nc(B, T, C)
    in_maps = prep_inputs(x, Wq, Wk, Wv, Wo, bo)
    res = run_bass_kernel_spmd(
        nc, in_maps, core_ids=list(range(N_CORES)), trace=_trace
    )
    TH = T // (2 * N_CORES)  # 128: core m holds tokens {e*T/2 + m*TH + t}
    CO = np.asarray(Wo).shape[1]
    out = np.empty((B, T, CO), dtype=np.float32)
    for m in range(N_CORES):
        r = res.results[m]["out"]  # [B, 2*TH, CO]
        for e in range(2):
            g0 = e * (T // 2) + m * TH
            out[:, g0:g0 + TH, :] = r[:, e * TH:(e + 1) * TH, :]
    if _trace:
        kernel.last_result = res
    return out
